# revision 4
# baseline (speedup 1.0000x reference)
"""CostVolume kernel for Trainium2 (8 NeuronCores, Bass/Tile).

Math: the reference computes a 9x9-displacement correlation cost volume and
scatters it into out[b, r', c', r, c].  Substituting r' = r + di - 4,
c' = c + dj - 4 shows the output is just a banded Gram matrix:

    out[b, r', c', r, c] = (sum_ch feat2[b,ch,r',c'] * feat1[b,ch,r,c])
                           * 1[|r'-r| <= 4] * 1[|c'-c| <= 4]

so the kernel is: per batch, a (H*W x H*W) Gram matrix restricted to the
9-row band (computed as TensorEngine matmuls), a constant mask multiply,
and dense writes (mostly zeros) of the (H*W, H, W) output.

Sharding: 8 cores = 4 batches x 2 column-halves (c' in [0,32) / [32,64)).
Column sharding keeps the row-edge structure identical on every core, so a
single SPMD program serves all 8 cores; only the data (feat2 column slice
+ the c'-band mask) differs per core.

Per core: 16 "quads" (4 consecutive r' rows x 32 c' = 128 PSUM partitions).
Quad k computes psum[128, 704] = f2_quad[256,128]^T @ f1_window[256,704]
(f1 window = rows 4k-4 .. 4k+6, zero-padded at the image edges), applies
the band mask on the Vector engine, and writes its 2 MiB output chunk with
three DMAs: zero prefix rows, the 704-column band, zero suffix rows.
"""

import numpy as np

B, C, H, W = 4, 256, 64, 64
MD = 4
N_CORES = 8
CSH = W // 2          # 32 c' columns per core
RQ = 4                # r' rows per quad
NQ = H // RQ          # 16 quads
RB = 2 * MD + RQ      # 12 r-blocks in a quad's band window (r0-4 .. r0+7)
NW = RB * W           # 768 band columns

_COMPILED = None  # (nc, ) cache so repeated kernel() calls skip rebuild


def _build_program():
    import concourse.bacc as bacc
    import concourse.tile as tile
    from concourse import mybir

    f32 = mybir.dt.float32
    nc = bacc.Bacc("TRN2", target_bir_lowering=False, debug=False,
                   num_devices=N_CORES)

    # DRAM I/O (per-core shard shapes)
    f2s = nc.dram_tensor("f2s", [C, H * CSH], f32, kind="ExternalInput").ap()
    f1p = nc.dram_tensor("f1p", [C, (H + 2 * MD) * W], f32,
                         kind="ExternalInput").ap()
    msk = nc.dram_tensor("msk", [128, NW], f32, kind="ExternalInput").ap()
    out = nc.dram_tensor("out", [H * CSH, H * W], f32,
                         kind="ExternalOutput").ap()

    max_zero = 0
    for k in range(NQ):
        r0 = RQ * k
        wlo = max(0, r0 - MD)
        whi = min(H, r0 + MD + RQ)
        max_zero = max(max_zero, wlo, H - whi)

    with tile.TileContext(nc) as tc:
        with (
            tc.tile_pool(name="persist", bufs=1) as persist,
            tc.tile_pool(name="band", bufs=3) as band_pool,
            tc.tile_pool(name="psum", bufs=2, space="PSUM") as psum_pool,
        ):
            # resident inputs
            f2_t = []
            f1_t = []
            for h in range(2):
                t2 = persist.tile([128, H * CSH], f32, tag=f"f2_{h}")
                nc.sync.dma_start(out=t2[:], in_=f2s[h * 128:(h + 1) * 128, :])
                f2_t.append(t2)
                t1 = persist.tile([128, (H + 2 * MD) * W], f32, tag=f"f1_{h}")
                nc.sync.dma_start(out=t1[:], in_=f1p[h * 128:(h + 1) * 128, :])
                f1_t.append(t1)
            mask_t = persist.tile([128, NW], f32, tag="mask")
            nc.sync.dma_start(out=mask_t[:], in_=msk[:])
            zero_t = persist.tile([128, max_zero * W], f32, tag="zeros")
            nc.vector.memset(zero_t[:], 0.0)

            for k in range(NQ):
                r0 = RQ * k
                wlo = max(0, r0 - MD)       # first valid r row written
                whi = min(H, r0 + MD + RQ)  # one past last valid r row
                a = wlo - (r0 - MD)             # valid start block in window
                b = whi - (r0 - MD)

                psum = psum_pool.tile([128, NW], f32)
                for (n0, n1) in ((0, 512), (512, NW)):
                    nc.tensor.matmul(
                        psum[:, n0:n1],
                        f2_t[0][:, k * 128:(k + 1) * 128],
                        f1_t[0][:, r0 * W + n0: r0 * W + n1],
                        start=True, stop=False,
                    )
                    nc.tensor.matmul(
                        psum[:, n0:n1],
                        f2_t[1][:, k * 128:(k + 1) * 128],
                        f1_t[1][:, r0 * W + n0: r0 * W + n1],
                        start=False, stop=True,
                    )
                band = band_pool.tile([128, NW], f32)
                nc.vector.tensor_mul(band[:], psum[:], mask_t[:])

                rows = slice(k * 128, (k + 1) * 128)
                nc.sync.dma_start(out=out[rows, wlo * W:whi * W],
                                  in_=band[:, a * W:b * W])
                if wlo > 0:
                    nc.sync.dma_start(out=out[rows, 0:wlo * W],
                                      in_=zero_t[:, 0:wlo * W])
                if whi < H:
                    nc.sync.dma_start(out=out[rows, whi * W:H * W],
                                      in_=zero_t[:, 0:(H - whi) * W])

    nc.compile()
    return nc


def _shard_inputs(feat1, feat2):
    """Per-core input dicts. Core i = (batch i//2, column-half i%2)."""
    in_maps = []
    for i in range(N_CORES):
        b, ch = divmod(i, 2)
        clo = ch * CSH
        f2slice = np.ascontiguousarray(feat2[b, :, :, clo:clo + CSH])
        f1pad = np.zeros((C, H + 2 * MD, W), np.float32)
        f1pad[:, MD:MD + H, :] = feat1[b]
        p = np.arange(128)
        rg = (p // CSH)[:, None, None]
        cj = (clo + p % CSH)[:, None, None]
        blk = np.arange(RB)[None, :, None]
        cc = np.arange(W)[None, None, :]
        m = ((blk - rg >= 0) & (blk - rg <= 2 * MD)
             & (np.abs(cj - cc) <= MD)).astype(np.float32)
        in_maps.append({
            "f2s": f2slice.reshape(C, H * CSH),
            "f1p": f1pad.reshape(C, (H + 2 * MD) * W),
            "msk": m.reshape(128, NW),
        })
    return in_maps


def run(feat1, feat2, trace=False, trace_cores=None):
    """Returns (full output (B, H*W, H, W) float32, exec_time_ns or None)."""
    global _COMPILED
    from concourse.bass_utils import run_bass_kernel_spmd

    feat1 = np.asarray(feat1, dtype=np.float32)
    feat2 = np.asarray(feat2, dtype=np.float32)
    assert feat1.shape == (B, C, H, W) and feat2.shape == (B, C, H, W)

    if _COMPILED is None:
        _COMPILED = _build_program()
    nc = _COMPILED

    in_maps = _shard_inputs(feat1, feat2)
    res = run_bass_kernel_spmd(
        nc, in_maps, core_ids=list(range(N_CORES)),
        trace=trace, trace_cores=trace_cores,
    )

    out5 = np.empty((B, H, W, H, W), np.float32)
    for i in range(N_CORES):
        b, ch = divmod(i, 2)
        shard = res.results[i]["out"].reshape(H, CSH, H, W)
        out5[b, :, ch * CSH:(ch + 1) * CSH, :, :] = shard
    return out5.reshape(B, H * W, H, W), res.exec_time_ns


def kernel(feat1, feat2):
    out, _ = run(feat1, feat2, trace=False)
    return out


# revision 8
# speedup vs baseline: 1.1736x; 1.1736x over previous
"""CostVolume kernel for Trainium2 (8 NeuronCores, Bass/Tile).

Math: the reference computes a 9x9-displacement correlation cost volume and
scatters it into out[b, r', c', r, c].  Substituting r' = r + di - 4,
c' = c + dj - 4 shows the output is just a banded Gram matrix:

    out[b, r', c', r, c] = (sum_ch feat2[b,ch,r',c'] * feat1[b,ch,r,c])
                           * 1[|r'-r| <= 4] * 1[|c'-c| <= 4]

so the kernel is: per batch, a (H*W x H*W) Gram matrix restricted to the
9-row band (computed as TensorEngine matmuls), a constant mask multiply,
and dense writes (mostly zeros) of the (H*W, H, W) output.

Sharding: 8 cores = 4 batches x 2 column-halves (c' in [0,32) / [32,64)).
Column sharding keeps the row-edge structure identical on every core, so a
single SPMD program serves all 8 cores; only the data (feat2 column slice
+ the c'-band mask) differs per core.

Per core: 16 "quads" (4 consecutive r' rows x 32 c' = 128 PSUM partitions).
Quad k computes psum[128, 704] = f2_quad[256,128]^T @ f1_window[256,704]
(f1 window = rows 4k-4 .. 4k+6, zero-padded at the image edges), applies
the band mask on the Vector engine, and writes its 2 MiB output chunk with
three DMAs: zero prefix rows, the 704-column band, zero suffix rows.
"""

import numpy as np

B, C, H, W = 4, 256, 64, 64
MD = 4
N_CORES = 8
CSH = W // 2          # 32 c' columns per core
RQ = 4                # r' rows per quad
NQ = H // RQ          # 16 quads
RB = 2 * MD + RQ      # 12 r-blocks in a quad's band window (r0-4 .. r0+7)
NW = RB * W           # 768 band columns

USE_F32R = True   # float32r matmuls: 4x TensorE throughput, near-fp32 numerics

_COMPILED = None  # (nc, ) cache so repeated kernel() calls skip rebuild


def _build_program():
    import concourse.bacc as bacc
    import concourse.tile as tile
    from concourse import mybir

    f32 = mybir.dt.float32
    mm_dt = mybir.dt.float32r if USE_F32R else f32
    nc = bacc.Bacc("TRN2", target_bir_lowering=False, debug=False,
                   num_devices=N_CORES)

    # DRAM I/O (per-core shard shapes)
    f2s = nc.dram_tensor("f2s", [C, H * CSH], f32, kind="ExternalInput").ap()
    f1p = nc.dram_tensor("f1p", [C, (H + 2 * MD) * W], f32,
                         kind="ExternalInput").ap()
    msk = nc.dram_tensor("msk", [128, NW], f32, kind="ExternalInput").ap()
    out = nc.dram_tensor("out", [H * CSH, H * W], f32,
                         kind="ExternalOutput").ap()

    max_zero = 0
    for k in range(NQ):
        r0 = RQ * k
        wlo = max(0, r0 - MD)
        whi = min(H, r0 + MD + RQ)
        max_zero = max(max_zero, wlo, H - whi)

    with tile.TileContext(nc) as tc:
        with (
            tc.tile_pool(name="persist", bufs=1) as persist,
            tc.tile_pool(name="band", bufs=3) as band_pool,
            tc.tile_pool(name="psum", bufs=2, space="PSUM") as psum_pool,
        ):
            # resident inputs
            f2_t = []
            f1_t = []
            for h in range(2):
                t2 = persist.tile([128, H * CSH], mm_dt, tag=f"f2_{h}")
                nc.sync.dma_start(out=t2[:],
                                  in_=f2s[h * 128:(h + 1) * 128, :].bitcast(mm_dt))
                f2_t.append(t2)
                t1 = persist.tile([128, (H + 2 * MD) * W], mm_dt, tag=f"f1_{h}")
                nc.sync.dma_start(out=t1[:],
                                  in_=f1p[h * 128:(h + 1) * 128, :].bitcast(mm_dt))
                f1_t.append(t1)
            mask_t = persist.tile([128, NW], f32, tag="mask")
            nc.sync.dma_start(out=mask_t[:], in_=msk[:])
            zero_t = persist.tile([128, max_zero * W], f32, tag="zeros")
            nc.vector.memset(zero_t[:], 0.0)

            for k in range(NQ):
                r0 = RQ * k
                wlo = max(0, r0 - MD)       # first valid r row written
                whi = min(H, r0 + MD + RQ)  # one past last valid r row
                a = wlo - (r0 - MD)             # valid start block in window
                b = whi - (r0 - MD)

                psum = psum_pool.tile([128, NW], f32)
                for (n0, n1) in ((0, 512), (512, NW)):
                    nc.tensor.matmul(
                        psum[:, n0:n1],
                        f2_t[0][:, k * 128:(k + 1) * 128],
                        f1_t[0][:, r0 * W + n0: r0 * W + n1],
                        start=True, stop=False,
                    )
                    nc.tensor.matmul(
                        psum[:, n0:n1],
                        f2_t[1][:, k * 128:(k + 1) * 128],
                        f1_t[1][:, r0 * W + n0: r0 * W + n1],
                        start=False, stop=True,
                    )
                band = band_pool.tile([128, NW], f32)
                nc.vector.tensor_mul(band[:], psum[:], mask_t[:])

                rows = slice(k * 128, (k + 1) * 128)
                nc.sync.dma_start(out=out[rows, wlo * W:whi * W],
                                  in_=band[:, a * W:b * W])
                if wlo > 0:
                    nc.sync.dma_start(out=out[rows, 0:wlo * W],
                                      in_=zero_t[:, 0:wlo * W])
                if whi < H:
                    nc.sync.dma_start(out=out[rows, whi * W:H * W],
                                      in_=zero_t[:, 0:(H - whi) * W])

    nc.compile()
    return nc


def _shard_inputs(feat1, feat2):
    """Per-core input dicts. Core i = (batch i//2, column-half i%2)."""
    in_maps = []
    for i in range(N_CORES):
        b, ch = divmod(i, 2)
        clo = ch * CSH
        f2slice = np.ascontiguousarray(feat2[b, :, :, clo:clo + CSH])
        f1pad = np.zeros((C, H + 2 * MD, W), np.float32)
        f1pad[:, MD:MD + H, :] = feat1[b]
        p = np.arange(128)
        rg = (p // CSH)[:, None, None]
        cj = (clo + p % CSH)[:, None, None]
        blk = np.arange(RB)[None, :, None]
        cc = np.arange(W)[None, None, :]
        m = ((blk - rg >= 0) & (blk - rg <= 2 * MD)
             & (np.abs(cj - cc) <= MD)).astype(np.float32)
        in_maps.append({
            "f2s": f2slice.reshape(C, H * CSH),
            "f1p": f1pad.reshape(C, (H + 2 * MD) * W),
            "msk": m.reshape(128, NW),
        })
    return in_maps


def run(feat1, feat2, trace=False, trace_cores=None):
    """Returns (full output (B, H*W, H, W) float32, exec_time_ns or None)."""
    global _COMPILED
    from concourse.bass_utils import run_bass_kernel_spmd

    feat1 = np.asarray(feat1, dtype=np.float32)
    feat2 = np.asarray(feat2, dtype=np.float32)
    assert feat1.shape == (B, C, H, W) and feat2.shape == (B, C, H, W)

    if _COMPILED is None:
        _COMPILED = _build_program()
    nc = _COMPILED

    in_maps = _shard_inputs(feat1, feat2)
    res = run_bass_kernel_spmd(
        nc, in_maps, core_ids=list(range(N_CORES)),
        trace=trace, trace_cores=trace_cores,
    )

    out5 = np.empty((B, H, W, H, W), np.float32)
    for i in range(N_CORES):
        b, ch = divmod(i, 2)
        shard = res.results[i]["out"].reshape(H, CSH, H, W)
        out5[b, :, ch * CSH:(ch + 1) * CSH, :, :] = shard
    return out5.reshape(B, H * W, H, W), res.exec_time_ns


def kernel(feat1, feat2):
    out, _ = run(feat1, feat2, trace=False)
    return out
